# revision 35
# baseline (speedup 1.0000x reference)
"""MoE gating kernel (logits -> softmax -> top-2 mask) for 8 trn2 NeuronCores.

Math: logits = x @ W.T + b  [B,S,E]; weights = softmax(logits, -1);
gated = weights masked to per-token top-2.  Returns (gated.T, weights.T),
both [E, B, S] fp32.

Strategy (v15 = v10 hi-plane only + on-device exact repair):
  - v10 streamed a full fp16 hi/lo split of x (32 MB/core) so the top-2
    selection is exact everywhere.  But only tokens whose 2nd/3rd logit
    gap is < ~1e-3 can mis-select under hi-plane-only error (sigma
    ~1.4e-4); that's ~50 tokens per core on N(0,1) data.
  - Phase 1 streams ONLY the hi plane A = fp16(x) (16 MB/core, half the
    DMA bytes and half the PE moving passes of v10):
        logits*2^8 ~= A@C.T + A@Dp.T,  C = fp16(W*2^8), Dp = fp16(W*2^8-C)
    softmax + top-2 exactly as v10.  Weights err ~1e-4 << 2e-2 tol;
    gated can be wrong only where flagged ambiguous.
  - Per group (overlapped under the stream), tokens with max8 2nd-3rd
    gap < TAU*2^8 are flagged and COMPACTED with no data-dependent
    control flow: an intra-partition Hillis-Steele scan plus a
    strict-lower-triangular-ones matmul gives each flagged token a
    per-group rank r < 32; a one-hot is_equal against an iota grid,
    multiplied by (token_id+1) and reduced, drops id+1 into compaction
    slot [g, r] of cid_acc[128, 8, 32] (at most one nonzero per slot).
  - Phase 2 (tail): two fp32 ones-matmuls collapse cid_acc across
    partitions into a 256-slot id list (0 = empty); (id+1)-1 feeds
    per-partition-offset gpsimd indirect row gathers of the flagged
    tokens' token-major A and B = 2^11(x-A) rows (out-of-bounds skipped);
    PE transposes restore d-major and v10's exact 3-term matmul + softmax
    recomputes those <=256 tokens; fix_out/fix_ids go to DRAM and the
    host overwrites gated[:, ids] (pure indexing).
  - atok/btok (token-major planes) are uploaded but only the ~50
    gathered rows are ever read, so they cost upload time, not HW time.
"""

import functools

import numpy as np

NUM_CORES = 8
TOK_PER_CORE = 8192
GROUPS = 8
GTOK = 1024
TILES = 8
CHUNKS = 8
D = 1024
E = 16
RCAP = 32  # repair slots per group (mean ~6 flagged; P(>32) ~ 1e-12)
NSLOT = GROUPS * RCAP  # 256

XS = 11  # x = A + 2^-XS * B
WS = 8  # accumulating logits * 2^WS
TAU = 2.5e-3  # ambiguity threshold on the 2nd/3rd logit gap (unscaled)

TRACE = False
LAST_RESULTS = None


@functools.lru_cache(maxsize=2)
def _build(has_b: bool):
    from concourse import bacc, mybir
    import concourse.bass as bass
    import concourse.tile as tile
    from concourse.masks import make_identity

    f16 = mybir.dt.float16
    f32 = mybir.dt.float32
    u32 = mybir.dt.uint32
    Exp = mybir.ActivationFunctionType.Exp
    Op = mybir.AluOpType
    X = mybir.AxisListType.X

    nc = bacc.Bacc(
        "TRN2", target_bir_lowering=False, debug=False, num_devices=NUM_CORES
    )

    # A.T shard: [1024 d, 8192 t] fp16, d-major (streamed)
    at_dram = nc.dram_tensor("a_t", [D, TOK_PER_CORE], f16, kind="ExternalInput").ap()
    # token-major planes for the repair gather (only ~50 rows ever read)
    atok_dram = nc.dram_tensor("atok", [TOK_PER_CORE, D], f16, kind="ExternalInput").ap()
    btok_dram = nc.dram_tensor("btok", [TOK_PER_CORE, D], f16, kind="ExternalInput").ap()
    cda_dram = nc.dram_tensor("cda", [128, CHUNKS, 4 * E], f16, kind="ExternalInput").ap()
    cs_dram = nc.dram_tensor("cs", [128, CHUNKS, E], f16, kind="ExternalInput").ap()
    tri_dram = nc.dram_tensor("tri", [128, 128], f16, kind="ExternalInput").ap()
    ids1_dram = nc.dram_tensor("ids1", [128, GROUPS * TILES], f32, kind="ExternalInput").ap()
    iota_dram = nc.dram_tensor("iota", [128, RCAP], f32, kind="ExternalInput").ap()
    if has_b:
        bcd_dram = nc.dram_tensor("bcd", [1, 4 * E], f16, kind="ExternalInput").ap()
    wts_dram = nc.dram_tensor("wts", [E, TOK_PER_CORE], f32, kind="ExternalOutput")
    gated_dram = nc.dram_tensor("gated", [E, TOK_PER_CORE], f32, kind="ExternalOutput")
    fixids_dram = nc.dram_tensor("fix_ids", [1, NSLOT], f32, kind="ExternalOutput")
    fixout_dram = nc.dram_tensor("fix_out", [NSLOT, E], f32, kind="ExternalOutput")

    def bcast_inner(ap, n):
        return bass.AP(tensor=ap.tensor, offset=ap.offset, ap=[*ap.ap, [0, n]])

    with tile.TileContext(nc) as tc:
        with (
            tc.tile_pool(name="consts", bufs=1) as consts,
            tc.tile_pool(name="xt", bufs=3) as xt_pool,
            tc.tile_pool(name="lg", bufs=2) as lg_pool,
            tc.tile_pool(name="sm", bufs=2) as sm_pool,
            tc.tile_pool(name="oacc", bufs=1) as oacc_pool,
            tc.tile_pool(name="rep", bufs=1) as rep_pool,
            tc.tile_pool(name="pss", bufs=5, space="PSUM") as pss_pool,
            tc.tile_pool(name="pslgt", bufs=2, space="PSUM") as pslgt_pool,
            tc.tile_pool(name="psout", bufs=1, space="PSUM") as psout_pool,
        ):
            cda_sb = consts.tile([128, CHUNKS, 4 * E], f16)
            cs_sb = consts.tile([128, CHUNKS, E], f16)
            tri_sb = consts.tile([128, 128], f16)
            ids1_sb = consts.tile([128, GROUPS * TILES], f32)
            iota_sb = consts.tile([128, RCAP], f32)
            nc.sync.dma_start(out=cda_sb, in_=cda_dram)
            nc.sync.dma_start(out=cs_sb, in_=cs_dram)
            nc.sync.dma_start(out=tri_sb, in_=tri_dram)
            nc.sync.dma_start(out=ids1_sb, in_=ids1_dram)
            nc.sync.dma_start(out=iota_sb, in_=iota_dram)
            ident32 = consts.tile([128, 128], f32)
            make_identity(nc, ident32)
            ident16 = consts.tile([128, 128], f16)
            make_identity(nc, ident16)
            ones32 = consts.tile([128, 1], f32)
            nc.vector.memset(ones32, 1.0)
            if has_b:
                bcd_sb = consts.tile([1, 4 * E], f16)
                nc.sync.dma_start(out=bcd_sb, in_=bcd_dram)
                ones_sb = consts.tile([1, 512], f16)
                nc.vector.memset(ones_sb, 1.0)

            w_acc = oacc_pool.tile([128, GROUPS, 128], f32)
            g_acc = oacc_pool.tile([128, GROUPS, 128], f32)
            flg_acc = oacc_pool.tile([128, GROUPS, TILES], f32)

            def mm_phase(g):
                xt_a = xt_pool.tile([128, CHUNKS, GTOK], f16, tag="xta")
                gs = slice(g * GTOK, (g + 1) * GTOK)
                # split loads per 2-chunk piece so matmul k can start as
                # soon as its chunks land (fine completion granularity)
                for k0 in (0, 2, 4, 6):
                    ksl = slice(k0 * 128, (k0 + 2) * 128)
                    nc.sync.dma_start(
                        out=xt_a[:, k0 : k0 + 2, :],
                        in_=at_dram[ksl, gs].rearrange("(k p) t -> p k t", p=128),
                    )

                s_h = [
                    pss_pool.tile([128, 512], f32, tag="s", name=f"s_g{g}h{h}")
                    for h in range(2)
                ]
                for k in range(CHUNKS):
                    last = k == CHUNKS - 1
                    for h in range(2):
                        ra = xt_a[:, k, 512 * h : 512 * (h + 1)]
                        nc.tensor.matmul(
                            s_h[h][0:64, :], lhsT=cda_sb[:, k, :], rhs=ra,
                            start=(k == 0), stop=(last and not has_b),
                            tile_position=(0, 0),
                        )
                if has_b:
                    for h in range(2):
                        nc.tensor.matmul(
                            s_h[h][0:64, :], lhsT=bcd_sb, rhs=ones_sb,
                            start=False, stop=True, tile_position=(0, 0),
                        )
                return s_h

            def tail_phase(g, s_h):
                # logits*2^8 = strip0 + strip32 (one PSUM input/op);
                # separate per-half tiles so tiles 0-3's transposes only
                # wait on half 0's add, not the whole group's combine
                lgH = []
                for h in range(2):
                    cmb = sm_pool.tile([E, 512], f32, tag="cmb")
                    nc.scalar.copy(cmb, s_h[h][0:16, :])
                    lgh = lg_pool.tile([E, 512], f32, tag=f"lgS{h}",
                                       name=f"lgS{g}_{h}")
                    nc.vector.tensor_add(lgh, cmb, s_h[h][32:48, :])
                    lgH.append(lgh)

                lgt_ps = pslgt_pool.tile([128, TILES, E], f32, tag="lgt_ps")
                for i in range(TILES):
                    nc.tensor.transpose(
                        lgt_ps[:, i, :],
                        lgH[i // 4][:, 128 * (i % 4) : 128 * (i % 4 + 1)],
                        ident32[:E, :E],
                    )
                lgt = sm_pool.tile([128, TILES, E], f32, tag="lgt")
                nc.vector.tensor_copy(lgt, lgt_ps)

                m8 = sm_pool.tile([128, TILES, 8], f32, tag="m8")
                for i in range(TILES):
                    nc.vector.max(m8[:, i, :], lgt[:, i, :])

                # ---- ambiguity flag (compaction itself is batched later) ----
                d23 = sm_pool.tile([128, TILES], f32, tag="d23")
                nc.vector.tensor_tensor(
                    out=d23, in0=m8[:, :, 1], in1=m8[:, :, 2], op=Op.subtract
                )
                nc.vector.tensor_scalar(
                    out=flg_acc[:, g, :], in0=d23,
                    scalar1=float(TAU * 2.0**WS), scalar2=None, op0=Op.is_lt,
                )

                # ---- softmax / top-2 / outputs (v10) ----
                ex = sm_pool.tile([128, TILES, E], f32, tag="ex")
                nc.scalar.activation(ex, lgt, func=Exp, scale=float(2.0**-WS))
                ssum = sm_pool.tile([128, TILES], f32, tag="ssum")
                nc.vector.tensor_reduce(ssum, ex, axis=X, op=Op.add)
                rec = sm_pool.tile([128, TILES], f32, tag="rec")
                nc.vector.reciprocal(rec, ssum)
                w_grp = sm_pool.tile([128, TILES, E], f32, tag="wg")
                nc.vector.tensor_tensor(
                    out=w_grp, in0=ex, in1=bcast_inner(rec[:, :], E), op=Op.mult
                )
                msk = sm_pool.tile([128, TILES, E], f32, tag="msk")
                nc.vector.tensor_tensor(
                    out=msk, in0=lgt, in1=bcast_inner(m8[:, :, 1], E), op=Op.is_ge
                )
                g_grp = sm_pool.tile([128, TILES, E], f32, tag="gg")
                nc.vector.tensor_tensor(out=g_grp, in0=msk, in1=w_grp, op=Op.mult)

                ps_o = psout_pool.tile([128, 256], f32, tag="ps_o")
                nc.tensor.transpose(ps_o[:, 0:128], w_grp, ident32)
                nc.tensor.transpose(ps_o[:, 128:256], g_grp, ident32)
                nc.scalar.copy(w_acc[:, g, :], ps_o[:, 0:128])
                nc.vector.tensor_copy(g_acc[:, g, :], ps_o[:, 128:256])

            # software pipeline: group g's matmuls, then group g-1's tail
            prev = None
            for g in range(GROUPS):
                s_h = mm_phase(g)
                if prev is not None:
                    tail_phase(prev[0], prev[1])
                prev = (g, s_h)
            tail_phase(prev[0], prev[1])

            # writeback: partition p=(tile,e); addr = e*8192 + g*1024 + tile*128 + t
            out_ap = [[128, TILES], [TOK_PER_CORE, E], [GTOK, GROUPS], [1, 128]]
            nc.sync.dma_start(
                out=bass.AP(tensor=wts_dram, offset=0, ap=list(out_ap)), in_=w_acc
            )
            nc.sync.dma_start(
                out=bass.AP(tensor=gated_dram, offset=0, ap=list(out_ap)), in_=g_acc
            )

            # ---- phase 2: batched compaction over all groups ----
            # segmented (per-group) inclusive scan along the 8 tile columns
            cur = flg_acc
            for k in (1, 2, 4):
                nxt = rep_pool.tile([128, GROUPS, TILES], f32, tag=f"bscan{k}")
                nc.vector.tensor_copy(nxt[:, :, 0:k], cur[:, :, 0:k])
                nc.vector.tensor_add(
                    nxt[:, :, k:TILES], cur[:, :, k:TILES], cur[:, :, 0 : TILES - k]
                )
                cur = nxt
            excl = rep_pool.tile([128, GROUPS, TILES], f32, tag="excl")
            nc.vector.tensor_tensor(out=excl, in0=cur, in1=flg_acc, op=Op.subtract)
            # per-group cross-partition exclusive prefix (one F=8 matmul)
            rsum16 = rep_pool.tile([128, GROUPS], f16, tag="rsum16")
            nc.vector.tensor_copy(rsum16, cur[:, :, TILES - 1])
            crps = pslgt_pool.tile([128, TILES, E], f32, tag="lgt_ps", name="crps")
            nc.tensor.matmul(crps[:, 0, 0:GROUPS], lhsT=tri_sb, rhs=rsum16,
                             start=True, stop=True)
            cross = rep_pool.tile([128, GROUPS], f32, tag="cross")
            nc.scalar.copy(cross, crps[:, 0, 0:GROUPS])
            # rank (flagged: < RCAP) or big (unflagged/overflow: no match)
            rank = rep_pool.tile([128, GROUPS, TILES], f32, tag="rank")
            nc.vector.tensor_tensor(
                out=rank, in0=excl, in1=bcast_inner(cross[:, :], TILES), op=Op.add
            )
            big = rep_pool.tile([128, GROUPS, TILES], f32, tag="big")
            nc.vector.tensor_scalar(
                out=big, in0=flg_acc, scalar1=-4096.0, scalar2=4096.0,
                op0=Op.mult, op1=Op.add,
            )
            nc.vector.tensor_add(rank, rank, big)
            # one-hot over r, times (token_id+1), reduced over tile columns
            oh = rep_pool.tile([128, GROUPS, RCAP, TILES], f32, tag="oh")
            nc.vector.tensor_tensor(
                out=oh,
                in0=bass.AP(tensor=rank.tensor, offset=rank.offset,
                            ap=[rank.ap[0], [TILES, GROUPS], [0, RCAP], [1, TILES]]),
                in1=bass.AP(tensor=iota_sb.tensor, offset=iota_sb.offset,
                            ap=[iota_sb.ap[0], [0, GROUPS], [1, RCAP], [0, TILES]]),
                op=Op.is_equal,
            )
            cidm = rep_pool.tile([128, GROUPS, RCAP, TILES], f32, tag="cidm")
            nc.vector.tensor_tensor(
                out=cidm, in0=oh,
                in1=bass.AP(tensor=ids1_sb.tensor, offset=ids1_sb.offset,
                            ap=[ids1_sb.ap[0], [TILES, GROUPS], [0, RCAP], [1, TILES]]),
                op=Op.mult,
            )
            cid_acc = rep_pool.tile([128, GROUPS, RCAP], f32, tag="cid_acc")
            nc.vector.tensor_reduce(cid_acc, cidm, axis=X, op=Op.add)

            # ---- collapse slots, gather, exact recompute ----
            cidflat = cid_acc.rearrange("p g r -> p (g r)")
            pscl = pss_pool.tile([128, 512], f32, tag="s", name="pscl")
            nc.tensor.matmul(pscl[:, 0:1], lhsT=cidflat[:, 0:128], rhs=ones32,
                             start=True, stop=True)
            nc.tensor.matmul(pscl[:, 1:2], lhsT=cidflat[:, 128:256], rhs=ones32,
                             start=True, stop=True)
            cidL = rep_pool.tile([128, 2], f32, tag="cidL")
            nc.scalar.copy(cidL, pscl[:, 0:2])
            nc.sync.dma_start(
                out=bass.AP(tensor=fixids_dram, offset=0, ap=[[1, 128], [128, 2]]),
                in_=cidL,
            )
            idm1 = rep_pool.tile([128, 2], f32, tag="idm1")
            nc.vector.tensor_scalar(
                out=idm1, in0=cidL, scalar1=-1.0, scalar2=None, op0=Op.add
            )
            idu = rep_pool.tile([128, 2], u32, tag="idu")
            nc.vector.tensor_copy(idu, idm1)

            # per-partition-offset row gathers; empty slots (-1 -> huge) skipped
            xra = rep_pool.tile([128, 2, D], f16, tag="xra")
            xrb = rep_pool.tile([128, 2, D], f16, tag="xrb")
            for c in range(2):
                nc.gpsimd.indirect_dma_start(
                    out=xra[:, c, :], out_offset=None, in_=atok_dram,
                    in_offset=bass.IndirectOffsetOnAxis(ap=idu[:, c : c + 1], axis=0),
                    bounds_check=TOK_PER_CORE - 1, oob_is_err=False,
                )
                nc.gpsimd.indirect_dma_start(
                    out=xrb[:, c, :], out_offset=None, in_=btok_dram,
                    in_offset=bass.IndirectOffsetOnAxis(ap=idu[:, c : c + 1], axis=0),
                    bounds_check=TOK_PER_CORE - 1, oob_is_err=False,
                )

            # token-major -> d-major via f16 PE transposes (no f32 roundtrip)
            xta = rep_pool.tile([128, CHUNKS, NSLOT], f16, tag="xta2")
            xtb = rep_pool.tile([128, CHUNKS, NSLOT], f16, tag="xtb2")
            for srcg, dst in ((xra, xta), (xrb, xtb)):
                for c in range(2):
                    for k in range(CHUNKS):
                        pst = pss_pool.tile([128, 512], f16, tag="s",
                                            name=f"pst{c}_{k}")
                        nc.tensor.transpose(
                            pst[:, 0:128],
                            srcg[:, c, 128 * k : 128 * (k + 1)], ident16,
                        )
                        nc.vector.tensor_copy(
                            dst[:, k, 128 * c : 128 * (c + 1)], pst[:, 0:128]
                        )

            # exact 3-term matmul for the repaired tokens (v10 math)
            s2 = pss_pool.tile([128, 512], f32, tag="s", name="s2rep")
            for k in range(CHUNKS):
                last = k == CHUNKS - 1
                nc.tensor.matmul(
                    s2[0:64, 0:NSLOT], lhsT=cda_sb[:, k, :], rhs=xta[:, k, :],
                    start=(k == 0), stop=(last and not has_b), tile_position=(0, 0),
                )
                nc.tensor.matmul(
                    s2[64:80, 0:NSLOT], lhsT=cs_sb[:, k, :], rhs=xtb[:, k, :],
                    start=(k == 0), stop=(last and not has_b), tile_position=(0, 64),
                )
            if has_b:
                nc.tensor.matmul(
                    s2[0:64, 0:NSLOT], lhsT=bcd_sb, rhs=ones_sb[:, 0:NSLOT],
                    start=False, stop=True, tile_position=(0, 0),
                )
                nc.tensor.matmul(
                    s2[64:80, 0:NSLOT], lhsT=cs_sb[0:1, 0, :], rhs=ones_sb[:, 0:NSLOT],
                    start=False, stop=True, tile_position=(0, 64),
                    skip_group_check=True,
                )

            c2 = rep_pool.tile([E, NSLOT], f32, tag="c2")
            nc.scalar.copy(c2, s2[0:16, 0:NSLOT])
            l2 = rep_pool.tile([E, NSLOT], f32, tag="l2")
            nc.vector.tensor_add(l2, c2, s2[32:48, 0:NSLOT])
            lr2 = rep_pool.tile([E, NSLOT], f32, tag="lr2")
            nc.vector.tensor_add(lr2, l2, s2[64:80, 0:NSLOT])

            psl = pslgt_pool.tile([128, TILES, E], f32, tag="lgt_ps", name="pslrep")
            for c in range(2):
                nc.tensor.transpose(
                    psl[:, c, :], lr2[:, 128 * c : 128 * (c + 1)], ident32[:E, :E]
                )
            lgr = rep_pool.tile([128, 2, E], f32, tag="lgr")
            nc.vector.tensor_copy(lgr, psl[:, 0:2, :])

            m8r = rep_pool.tile([128, 2, 8], f32, tag="m8r")
            for c in range(2):
                nc.vector.max(m8r[:, c, :], lgr[:, c, :])
            exr = rep_pool.tile([128, 2, E], f32, tag="exr")
            nc.scalar.activation(exr, lgr, func=Exp, scale=float(2.0**-WS))
            ssr = rep_pool.tile([128, 2], f32, tag="ssr")
            nc.vector.tensor_reduce(ssr, exr, axis=X, op=Op.add)
            rcr = rep_pool.tile([128, 2], f32, tag="rcr")
            nc.vector.reciprocal(rcr, ssr)
            wr = rep_pool.tile([128, 2, E], f32, tag="wr")
            nc.vector.tensor_tensor(
                out=wr, in0=exr, in1=bcast_inner(rcr[:, :], E), op=Op.mult
            )
            mkr = rep_pool.tile([128, 2, E], f32, tag="mkr")
            nc.vector.tensor_tensor(
                out=mkr, in0=lgr, in1=bcast_inner(m8r[:, :, 1], E), op=Op.is_ge
            )
            gr = rep_pool.tile([128, 2, E], f32, tag="gr")
            nc.vector.tensor_tensor(out=gr, in0=mkr, in1=wr, op=Op.mult)
            nc.sync.dma_start(
                out=bass.AP(tensor=fixout_dram, offset=0,
                            ap=[[E, 128], [128 * E, 2], [1, E]]),
                in_=gr,
            )

    nc.compile()
    return nc


def _w_consts(W):
    C = (W * np.float32(2.0**WS)).astype(np.float16)
    Dp = ((W - C.astype(np.float32) * np.float32(2.0**-WS)) * np.float32(2.0**WS)).astype(np.float16)
    Cs = (C.astype(np.float32) * np.float32(2.0**-XS)).astype(np.float16)

    def lay(M):  # [16, 1024] -> [128 d_lo, chunks, E]
        return np.ascontiguousarray(M.T.reshape(CHUNKS, 128, E).transpose(1, 0, 2))

    cda = np.zeros((128, CHUNKS, 4 * E), np.float16)
    cda[:, :, 0:E] = lay(C)
    cda[:, :, 2 * E : 3 * E] = lay(Dp)
    return cda, lay(Cs)


def kernel(x, W, b):
    global LAST_RESULTS
    from concourse.bass_utils import run_bass_kernel_spmd

    x = np.ascontiguousarray(np.asarray(x, dtype=np.float32))
    W = np.ascontiguousarray(np.asarray(W, dtype=np.float32))
    b = np.ascontiguousarray(np.asarray(b, dtype=np.float32))
    Bb, S, Dd = x.shape
    ntok = Bb * S
    assert (ntok, Dd) == (NUM_CORES * TOK_PER_CORE, D) and W.shape == (E, D)

    xf = x.reshape(ntok, D)
    A = xf.astype(np.float16)
    Bx = ((xf - A.astype(np.float32)) * np.float32(2.0**XS)).astype(np.float16)
    AT = np.ascontiguousarray(A.T)  # [1024, 65536]

    cda, cs = _w_consts(W)
    tri = np.fromfunction(lambda a, c: a < c, (128, 128)).astype(np.float16)
    j = np.arange(GROUPS * TILES)
    ids1 = ((j // TILES) * GTOK + (j % TILES) * 128)[None, :] + np.arange(128)[:, None]
    ids1 = (ids1 + 1).astype(np.float32)  # 0 stays "empty slot"
    iota = np.tile(np.arange(RCAP, dtype=np.float32), (128, 1))

    has_b = bool(np.any(b))
    in_maps = []
    for c in range(NUM_CORES):
        ts = slice(c * TOK_PER_CORE, (c + 1) * TOK_PER_CORE)
        m = {
            "a_t": np.ascontiguousarray(AT[:, ts]),
            "atok": np.ascontiguousarray(A[ts]),
            "btok": np.ascontiguousarray(Bx[ts]),
            "cda": cda,
            "cs": cs,
            "tri": tri,
            "ids1": ids1,
            "iota": iota,
        }
        if has_b:
            bc = (b * np.float32(2.0**WS)).astype(np.float16)
            bd = ((b - bc.astype(np.float32) * np.float32(2.0**-WS)) * np.float32(2.0**WS)).astype(np.float16)
            z = np.zeros(E, np.float16)
            m["bcd"] = np.concatenate([bc, z, bd, z]).reshape(1, 4 * E)
        in_maps.append(m)

    nc = _build(has_b)
    res = run_bass_kernel_spmd(
        nc, in_maps, core_ids=list(range(NUM_CORES)), trace=TRACE
    )
    LAST_RESULTS = res

    wts = np.concatenate([r["wts"] for r in res.results], axis=1)
    gated = np.concatenate([r["gated"] for r in res.results], axis=1)
    # apply the exact repairs (pure indexing); slot value = token_id+1, 0=empty
    for c, r in enumerate(res.results):
        ids = np.rint(r["fix_ids"].reshape(-1)).astype(np.int64) - 1
        fix = r["fix_out"]
        sel = ids >= 0
        if sel.any():
            tids = ids[sel]
            assert tids.max() < TOK_PER_CORE
            gated[:, c * TOK_PER_CORE + tids] = fix[sel].T
    return (
        gated.reshape(E, Bb, S).astype(np.float32),
        wts.reshape(E, Bb, S).astype(np.float32),
    )


# revision 37
# speedup vs baseline: 1.0452x; 1.0452x over previous
"""MoE gating kernel (logits -> softmax -> top-2 mask) for 8 trn2 NeuronCores.

Math: logits = x @ W.T + b  [B,S,E]; weights = softmax(logits, -1);
gated = weights masked to per-token top-2.  Returns (gated.T, weights.T),
both [E, B, S] fp32.

Strategy (v15 = v10 hi-plane only + on-device exact repair):
  - v10 streamed a full fp16 hi/lo split of x (32 MB/core) so the top-2
    selection is exact everywhere.  But only tokens whose 2nd/3rd logit
    gap is < ~1e-3 can mis-select under hi-plane-only error (sigma
    ~1.4e-4); that's ~50 tokens per core on N(0,1) data.
  - Phase 1 streams ONLY the hi plane A = fp16(x) (16 MB/core, half the
    DMA bytes and half the PE moving passes of v10):
        logits*2^8 ~= A@C.T + A@Dp.T,  C = fp16(W*2^8), Dp = fp16(W*2^8-C)
    softmax + top-2 exactly as v10.  Weights err ~1e-4 << 2e-2 tol;
    gated can be wrong only where flagged ambiguous.
  - Per group (overlapped under the stream), tokens with max8 2nd-3rd
    gap < TAU*2^8 are flagged and COMPACTED with no data-dependent
    control flow: an intra-partition Hillis-Steele scan plus a
    strict-lower-triangular-ones matmul gives each flagged token a
    per-group rank r < 32; a one-hot is_equal against an iota grid,
    multiplied by (token_id+1) and reduced, drops id+1 into compaction
    slot [g, r] of cid_acc[128, 8, 32] (at most one nonzero per slot).
  - Phase 2 (tail): two fp32 ones-matmuls collapse cid_acc across
    partitions into a 256-slot id list (0 = empty); (id+1)-1 feeds
    per-partition-offset gpsimd indirect row gathers of the flagged
    tokens' token-major A and B = 2^11(x-A) rows (out-of-bounds skipped);
    PE transposes restore d-major and v10's exact 3-term matmul + softmax
    recomputes those <=256 tokens; fix_out/fix_ids go to DRAM and the
    host overwrites gated[:, ids] (pure indexing).
  - atok/btok (token-major planes) are uploaded but only the ~50
    gathered rows are ever read, so they cost upload time, not HW time.
"""

import functools

import numpy as np

NUM_CORES = 8
TOK_PER_CORE = 8192
GROUPS = 8
GTOK = 1024
TILES = 8
CHUNKS = 8
D = 1024
E = 16
RCAP = 32  # repair slots per group (mean ~6 flagged; P(>32) ~ 1e-12)
NSLOT = GROUPS * RCAP  # 256

XS = 11  # x = A + 2^-XS * B
WS = 8  # accumulating logits * 2^WS
TAU = 2.5e-3  # ambiguity threshold on the 2nd/3rd logit gap (unscaled)

TRACE = False
LAST_RESULTS = None


@functools.lru_cache(maxsize=2)
def _build(has_b: bool):
    from concourse import bacc, mybir
    import concourse.bass as bass
    import concourse.tile as tile
    from concourse.masks import make_identity

    f16 = mybir.dt.float16
    f32 = mybir.dt.float32
    u32 = mybir.dt.uint32
    Exp = mybir.ActivationFunctionType.Exp
    Op = mybir.AluOpType
    X = mybir.AxisListType.X

    nc = bacc.Bacc(
        "TRN2", target_bir_lowering=False, debug=False, num_devices=NUM_CORES
    )

    # A.T shard: [1024 d, 8192 t] fp16, d-major (streamed)
    at_dram = nc.dram_tensor("a_t", [D, TOK_PER_CORE], f16, kind="ExternalInput").ap()
    # token-major planes for the repair gather (only ~50 rows ever read)
    atok_dram = nc.dram_tensor("atok", [TOK_PER_CORE, D], f16, kind="ExternalInput").ap()
    btok_dram = nc.dram_tensor("btok", [TOK_PER_CORE, D], f16, kind="ExternalInput").ap()
    cda_dram = nc.dram_tensor("cda", [128, CHUNKS, 4 * E], f16, kind="ExternalInput").ap()
    cs_dram = nc.dram_tensor("cs", [128, CHUNKS, E], f16, kind="ExternalInput").ap()
    tri_dram = nc.dram_tensor("tri", [128, 128], f16, kind="ExternalInput").ap()
    ids1_dram = nc.dram_tensor("ids1", [128, GROUPS * TILES], f32, kind="ExternalInput").ap()
    iota_dram = nc.dram_tensor("iota", [128, RCAP], f32, kind="ExternalInput").ap()
    if has_b:
        bcd_dram = nc.dram_tensor("bcd", [1, 4 * E], f16, kind="ExternalInput").ap()
    wts_dram = nc.dram_tensor("wts", [E, TOK_PER_CORE], f32, kind="ExternalOutput")
    gated_dram = nc.dram_tensor("gated", [E, TOK_PER_CORE], f32, kind="ExternalOutput")
    fixids_dram = nc.dram_tensor("fix_ids", [1, NSLOT], f32, kind="ExternalOutput")
    fixout_dram = nc.dram_tensor("fix_out", [NSLOT, E], f32, kind="ExternalOutput")

    def bcast_inner(ap, n):
        return bass.AP(tensor=ap.tensor, offset=ap.offset, ap=[*ap.ap, [0, n]])

    with tile.TileContext(nc) as tc:
        with (
            tc.tile_pool(name="consts", bufs=1) as consts,
            tc.tile_pool(name="xt", bufs=3) as xt_pool,
            tc.tile_pool(name="lg", bufs=2) as lg_pool,
            tc.tile_pool(name="sm", bufs=2) as sm_pool,
            tc.tile_pool(name="oacc", bufs=1) as oacc_pool,
            tc.tile_pool(name="rep", bufs=1) as rep_pool,
            tc.tile_pool(name="pss", bufs=5, space="PSUM") as pss_pool,
            tc.tile_pool(name="pslgt", bufs=2, space="PSUM") as pslgt_pool,
            tc.tile_pool(name="psout", bufs=1, space="PSUM") as psout_pool,
        ):
            cda_sb = consts.tile([128, CHUNKS, 4 * E], f16)
            cs_sb = consts.tile([128, CHUNKS, E], f16)
            tri_sb = consts.tile([128, 128], f16)
            ids1_sb = consts.tile([128, GROUPS * TILES], f32)
            iota_sb = consts.tile([128, RCAP], f32)
            # consts go via gpsimd SWDGE so the sync HWDGE queue opens with
            # the first x-chunk instead of five table loads (ramp -3-4us)
            nc.gpsimd.dma_start(out=cda_sb, in_=cda_dram)
            nc.gpsimd.dma_start(out=cs_sb, in_=cs_dram)
            nc.gpsimd.dma_start(out=tri_sb, in_=tri_dram)
            nc.gpsimd.dma_start(out=ids1_sb, in_=ids1_dram)
            nc.gpsimd.dma_start(out=iota_sb, in_=iota_dram)
            ident32 = consts.tile([128, 128], f32)
            make_identity(nc, ident32)
            ident16 = consts.tile([128, 128], f16)
            make_identity(nc, ident16)
            ones32 = consts.tile([128, 1], f32)
            nc.vector.memset(ones32, 1.0)
            if has_b:
                bcd_sb = consts.tile([1, 4 * E], f16)
                nc.sync.dma_start(out=bcd_sb, in_=bcd_dram)
                ones_sb = consts.tile([1, 512], f16)
                nc.vector.memset(ones_sb, 1.0)

            w_acc = oacc_pool.tile([128, GROUPS, 128], f32)
            g_acc = oacc_pool.tile([128, GROUPS, 128], f32)
            flg_acc = oacc_pool.tile([128, GROUPS, TILES], f32)

            def mm_phase(g):
                xt_a = xt_pool.tile([128, CHUNKS, GTOK], f16, tag="xta")
                gs = slice(g * GTOK, (g + 1) * GTOK)
                # split loads per 2-chunk piece so matmul k can start as
                # soon as its chunks land (fine completion granularity)
                for k0 in (0, 2, 4, 6):
                    ksl = slice(k0 * 128, (k0 + 2) * 128)
                    nc.sync.dma_start(
                        out=xt_a[:, k0 : k0 + 2, :],
                        in_=at_dram[ksl, gs].rearrange("(k p) t -> p k t", p=128),
                    )

                s_h = [
                    pss_pool.tile([128, 512], f32, tag="s", name=f"s_g{g}h{h}")
                    for h in range(2)
                ]
                for k in range(CHUNKS):
                    last = k == CHUNKS - 1
                    for h in range(2):
                        ra = xt_a[:, k, 512 * h : 512 * (h + 1)]
                        nc.tensor.matmul(
                            s_h[h][0:64, :], lhsT=cda_sb[:, k, :], rhs=ra,
                            start=(k == 0), stop=(last and not has_b),
                            tile_position=(0, 0),
                        )
                if has_b:
                    for h in range(2):
                        nc.tensor.matmul(
                            s_h[h][0:64, :], lhsT=bcd_sb, rhs=ones_sb,
                            start=False, stop=True, tile_position=(0, 0),
                        )
                return s_h

            def tail_phase(g, s_h):
                # logits*2^8 = strip0 + strip32 (one PSUM input/op)
                lgS = lg_pool.tile([E, GTOK], f32, tag="lgS", name=f"lgS{g}")
                for h in range(2):
                    cmb = sm_pool.tile([E, 512], f32, tag="cmb")
                    nc.scalar.copy(cmb, s_h[h][0:16, :])
                    nc.vector.tensor_add(
                        lgS[:, 512 * h : 512 * (h + 1)], cmb, s_h[h][32:48, :]
                    )

                lgt_ps = pslgt_pool.tile([128, TILES, E], f32, tag="lgt_ps")
                for i in range(TILES):
                    nc.tensor.transpose(
                        lgt_ps[:, i, :],
                        lgS[:, 128 * i : 128 * (i + 1)],
                        ident32[:E, :E],
                    )
                lgt = sm_pool.tile([128, TILES, E], f32, tag="lgt")
                nc.vector.tensor_copy(lgt, lgt_ps)

                m8 = sm_pool.tile([128, TILES, 8], f32, tag="m8")
                for i in range(TILES):
                    nc.vector.max(m8[:, i, :], lgt[:, i, :])

                # ---- ambiguity flag (compaction itself is batched later) ----
                d23 = sm_pool.tile([128, TILES], f32, tag="d23")
                nc.vector.tensor_tensor(
                    out=d23, in0=m8[:, :, 1], in1=m8[:, :, 2], op=Op.subtract
                )
                nc.vector.tensor_scalar(
                    out=flg_acc[:, g, :], in0=d23,
                    scalar1=float(TAU * 2.0**WS), scalar2=None, op0=Op.is_lt,
                )

                # ---- softmax / top-2 / outputs (v10) ----
                ex = sm_pool.tile([128, TILES, E], f32, tag="ex")
                nc.scalar.activation(ex, lgt, func=Exp, scale=float(2.0**-WS))
                ssum = sm_pool.tile([128, TILES], f32, tag="ssum")
                nc.vector.tensor_reduce(ssum, ex, axis=X, op=Op.add)
                rec = sm_pool.tile([128, TILES], f32, tag="rec")
                nc.vector.reciprocal(rec, ssum)
                w_grp = sm_pool.tile([128, TILES, E], f32, tag="wg")
                nc.vector.tensor_tensor(
                    out=w_grp, in0=ex, in1=bcast_inner(rec[:, :], E), op=Op.mult
                )
                msk = sm_pool.tile([128, TILES, E], f32, tag="msk")
                nc.vector.tensor_tensor(
                    out=msk, in0=lgt, in1=bcast_inner(m8[:, :, 1], E), op=Op.is_ge
                )
                g_grp = sm_pool.tile([128, TILES, E], f32, tag="gg")
                nc.vector.tensor_tensor(out=g_grp, in0=msk, in1=w_grp, op=Op.mult)

                ps_o = psout_pool.tile([128, 256], f32, tag="ps_o")
                nc.tensor.transpose(ps_o[:, 0:128], w_grp, ident32)
                nc.tensor.transpose(ps_o[:, 128:256], g_grp, ident32)
                nc.scalar.copy(w_acc[:, g, :], ps_o[:, 0:128])
                nc.vector.tensor_copy(g_acc[:, g, :], ps_o[:, 128:256])

            # software pipeline: group g's matmuls, then group g-1's tail
            prev = None
            for g in range(GROUPS):
                s_h = mm_phase(g)
                if prev is not None:
                    tail_phase(prev[0], prev[1])
                prev = (g, s_h)
            tail_phase(prev[0], prev[1])

            # writeback: partition p=(tile,e); addr = e*8192 + g*1024 + tile*128 + t
            out_ap = [[128, TILES], [TOK_PER_CORE, E], [GTOK, GROUPS], [1, 128]]
            nc.sync.dma_start(
                out=bass.AP(tensor=wts_dram, offset=0, ap=list(out_ap)), in_=w_acc
            )
            nc.sync.dma_start(
                out=bass.AP(tensor=gated_dram, offset=0, ap=list(out_ap)), in_=g_acc
            )

            # ---- phase 2: batched compaction over all groups ----
            # segmented (per-group) inclusive scan along the 8 tile columns
            cur = flg_acc
            for k in (1, 2, 4):
                nxt = rep_pool.tile([128, GROUPS, TILES], f32, tag=f"bscan{k}")
                nc.vector.tensor_copy(nxt[:, :, 0:k], cur[:, :, 0:k])
                nc.vector.tensor_add(
                    nxt[:, :, k:TILES], cur[:, :, k:TILES], cur[:, :, 0 : TILES - k]
                )
                cur = nxt
            excl = rep_pool.tile([128, GROUPS, TILES], f32, tag="excl")
            nc.vector.tensor_tensor(out=excl, in0=cur, in1=flg_acc, op=Op.subtract)
            # per-group cross-partition exclusive prefix (one F=8 matmul)
            rsum16 = rep_pool.tile([128, GROUPS], f16, tag="rsum16")
            nc.vector.tensor_copy(rsum16, cur[:, :, TILES - 1])
            crps = pslgt_pool.tile([128, TILES, E], f32, tag="lgt_ps", name="crps")
            nc.tensor.matmul(crps[:, 0, 0:GROUPS], lhsT=tri_sb, rhs=rsum16,
                             start=True, stop=True)
            cross = rep_pool.tile([128, GROUPS], f32, tag="cross")
            nc.scalar.copy(cross, crps[:, 0, 0:GROUPS])
            # rank (flagged: < RCAP) or big (unflagged/overflow: no match)
            rank = rep_pool.tile([128, GROUPS, TILES], f32, tag="rank")
            nc.vector.tensor_tensor(
                out=rank, in0=excl, in1=bcast_inner(cross[:, :], TILES), op=Op.add
            )
            big = rep_pool.tile([128, GROUPS, TILES], f32, tag="big")
            nc.vector.tensor_scalar(
                out=big, in0=flg_acc, scalar1=-4096.0, scalar2=4096.0,
                op0=Op.mult, op1=Op.add,
            )
            nc.vector.tensor_add(rank, rank, big)
            # one-hot over r, times (token_id+1), reduced over tile columns
            oh = rep_pool.tile([128, GROUPS, RCAP, TILES], f32, tag="oh")
            nc.vector.tensor_tensor(
                out=oh,
                in0=bass.AP(tensor=rank.tensor, offset=rank.offset,
                            ap=[rank.ap[0], [TILES, GROUPS], [0, RCAP], [1, TILES]]),
                in1=bass.AP(tensor=iota_sb.tensor, offset=iota_sb.offset,
                            ap=[iota_sb.ap[0], [0, GROUPS], [1, RCAP], [0, TILES]]),
                op=Op.is_equal,
            )
            cidm = rep_pool.tile([128, GROUPS, RCAP, TILES], f32, tag="cidm")
            nc.vector.tensor_tensor(
                out=cidm, in0=oh,
                in1=bass.AP(tensor=ids1_sb.tensor, offset=ids1_sb.offset,
                            ap=[ids1_sb.ap[0], [TILES, GROUPS], [0, RCAP], [1, TILES]]),
                op=Op.mult,
            )
            cid_acc = rep_pool.tile([128, GROUPS, RCAP], f32, tag="cid_acc")
            nc.vector.tensor_reduce(cid_acc, cidm, axis=X, op=Op.add)

            # ---- collapse slots, gather, exact recompute ----
            cidflat = cid_acc.rearrange("p g r -> p (g r)")
            pscl = pss_pool.tile([128, 512], f32, tag="s", name="pscl")
            nc.tensor.matmul(pscl[:, 0:1], lhsT=cidflat[:, 0:128], rhs=ones32,
                             start=True, stop=True)
            nc.tensor.matmul(pscl[:, 1:2], lhsT=cidflat[:, 128:256], rhs=ones32,
                             start=True, stop=True)
            cidL = rep_pool.tile([128, 2], f32, tag="cidL")
            nc.scalar.copy(cidL, pscl[:, 0:2])
            nc.sync.dma_start(
                out=bass.AP(tensor=fixids_dram, offset=0, ap=[[1, 128], [128, 2]]),
                in_=cidL,
            )
            idm1 = rep_pool.tile([128, 2], f32, tag="idm1")
            nc.vector.tensor_scalar(
                out=idm1, in0=cidL, scalar1=-1.0, scalar2=None, op0=Op.add
            )
            idu = rep_pool.tile([128, 2], u32, tag="idu")
            nc.vector.tensor_copy(idu, idm1)

            # per-partition-offset row gathers; empty slots (-1 -> huge) skipped
            xra = rep_pool.tile([128, 2, D], f16, tag="xra")
            xrb = rep_pool.tile([128, 2, D], f16, tag="xrb")
            for c in range(2):
                nc.gpsimd.indirect_dma_start(
                    out=xra[:, c, :], out_offset=None, in_=atok_dram,
                    in_offset=bass.IndirectOffsetOnAxis(ap=idu[:, c : c + 1], axis=0),
                    bounds_check=TOK_PER_CORE - 1, oob_is_err=False,
                )
                nc.gpsimd.indirect_dma_start(
                    out=xrb[:, c, :], out_offset=None, in_=btok_dram,
                    in_offset=bass.IndirectOffsetOnAxis(ap=idu[:, c : c + 1], axis=0),
                    bounds_check=TOK_PER_CORE - 1, oob_is_err=False,
                )

            # token-major -> d-major via f16 PE transposes (no f32 roundtrip)
            xta = rep_pool.tile([128, CHUNKS, NSLOT], f16, tag="xta2")
            xtb = rep_pool.tile([128, CHUNKS, NSLOT], f16, tag="xtb2")
            for srcg, dst in ((xra, xta), (xrb, xtb)):
                for c in range(2):
                    for k in range(CHUNKS):
                        pst = pss_pool.tile([128, 512], f16, tag="s",
                                            name=f"pst{c}_{k}")
                        nc.tensor.transpose(
                            pst[:, 0:128],
                            srcg[:, c, 128 * k : 128 * (k + 1)], ident16,
                        )
                        nc.vector.tensor_copy(
                            dst[:, k, 128 * c : 128 * (c + 1)], pst[:, 0:128]
                        )

            # exact 3-term matmul for the repaired tokens (v10 math)
            s2 = pss_pool.tile([128, 512], f32, tag="s", name="s2rep")
            for k in range(CHUNKS):
                last = k == CHUNKS - 1
                nc.tensor.matmul(
                    s2[0:64, 0:NSLOT], lhsT=cda_sb[:, k, :], rhs=xta[:, k, :],
                    start=(k == 0), stop=(last and not has_b), tile_position=(0, 0),
                )
                nc.tensor.matmul(
                    s2[64:80, 0:NSLOT], lhsT=cs_sb[:, k, :], rhs=xtb[:, k, :],
                    start=(k == 0), stop=(last and not has_b), tile_position=(0, 64),
                )
            if has_b:
                nc.tensor.matmul(
                    s2[0:64, 0:NSLOT], lhsT=bcd_sb, rhs=ones_sb[:, 0:NSLOT],
                    start=False, stop=True, tile_position=(0, 0),
                )
                nc.tensor.matmul(
                    s2[64:80, 0:NSLOT], lhsT=cs_sb[0:1, 0, :], rhs=ones_sb[:, 0:NSLOT],
                    start=False, stop=True, tile_position=(0, 64),
                    skip_group_check=True,
                )

            c2 = rep_pool.tile([E, NSLOT], f32, tag="c2")
            nc.scalar.copy(c2, s2[0:16, 0:NSLOT])
            l2 = rep_pool.tile([E, NSLOT], f32, tag="l2")
            nc.vector.tensor_add(l2, c2, s2[32:48, 0:NSLOT])
            lr2 = rep_pool.tile([E, NSLOT], f32, tag="lr2")
            nc.vector.tensor_add(lr2, l2, s2[64:80, 0:NSLOT])

            psl = pslgt_pool.tile([128, TILES, E], f32, tag="lgt_ps", name="pslrep")
            for c in range(2):
                nc.tensor.transpose(
                    psl[:, c, :], lr2[:, 128 * c : 128 * (c + 1)], ident32[:E, :E]
                )
            lgr = rep_pool.tile([128, 2, E], f32, tag="lgr")
            nc.vector.tensor_copy(lgr, psl[:, 0:2, :])

            m8r = rep_pool.tile([128, 2, 8], f32, tag="m8r")
            for c in range(2):
                nc.vector.max(m8r[:, c, :], lgr[:, c, :])
            exr = rep_pool.tile([128, 2, E], f32, tag="exr")
            nc.scalar.activation(exr, lgr, func=Exp, scale=float(2.0**-WS))
            ssr = rep_pool.tile([128, 2], f32, tag="ssr")
            nc.vector.tensor_reduce(ssr, exr, axis=X, op=Op.add)
            rcr = rep_pool.tile([128, 2], f32, tag="rcr")
            nc.vector.reciprocal(rcr, ssr)
            wr = rep_pool.tile([128, 2, E], f32, tag="wr")
            nc.vector.tensor_tensor(
                out=wr, in0=exr, in1=bcast_inner(rcr[:, :], E), op=Op.mult
            )
            mkr = rep_pool.tile([128, 2, E], f32, tag="mkr")
            nc.vector.tensor_tensor(
                out=mkr, in0=lgr, in1=bcast_inner(m8r[:, :, 1], E), op=Op.is_ge
            )
            gr = rep_pool.tile([128, 2, E], f32, tag="gr")
            nc.vector.tensor_tensor(out=gr, in0=mkr, in1=wr, op=Op.mult)
            nc.sync.dma_start(
                out=bass.AP(tensor=fixout_dram, offset=0,
                            ap=[[E, 128], [128 * E, 2], [1, E]]),
                in_=gr,
            )

    nc.compile()
    return nc


def _w_consts(W):
    C = (W * np.float32(2.0**WS)).astype(np.float16)
    Dp = ((W - C.astype(np.float32) * np.float32(2.0**-WS)) * np.float32(2.0**WS)).astype(np.float16)
    Cs = (C.astype(np.float32) * np.float32(2.0**-XS)).astype(np.float16)

    def lay(M):  # [16, 1024] -> [128 d_lo, chunks, E]
        return np.ascontiguousarray(M.T.reshape(CHUNKS, 128, E).transpose(1, 0, 2))

    cda = np.zeros((128, CHUNKS, 4 * E), np.float16)
    cda[:, :, 0:E] = lay(C)
    cda[:, :, 2 * E : 3 * E] = lay(Dp)
    return cda, lay(Cs)


def kernel(x, W, b):
    global LAST_RESULTS
    from concourse.bass_utils import run_bass_kernel_spmd

    x = np.ascontiguousarray(np.asarray(x, dtype=np.float32))
    W = np.ascontiguousarray(np.asarray(W, dtype=np.float32))
    b = np.ascontiguousarray(np.asarray(b, dtype=np.float32))
    Bb, S, Dd = x.shape
    ntok = Bb * S
    assert (ntok, Dd) == (NUM_CORES * TOK_PER_CORE, D) and W.shape == (E, D)

    xf = x.reshape(ntok, D)
    A = xf.astype(np.float16)
    Bx = ((xf - A.astype(np.float32)) * np.float32(2.0**XS)).astype(np.float16)
    AT = np.ascontiguousarray(A.T)  # [1024, 65536]

    cda, cs = _w_consts(W)
    tri = np.fromfunction(lambda a, c: a < c, (128, 128)).astype(np.float16)
    j = np.arange(GROUPS * TILES)
    ids1 = ((j // TILES) * GTOK + (j % TILES) * 128)[None, :] + np.arange(128)[:, None]
    ids1 = (ids1 + 1).astype(np.float32)  # 0 stays "empty slot"
    iota = np.tile(np.arange(RCAP, dtype=np.float32), (128, 1))

    has_b = bool(np.any(b))
    in_maps = []
    for c in range(NUM_CORES):
        ts = slice(c * TOK_PER_CORE, (c + 1) * TOK_PER_CORE)
        m = {
            "a_t": np.ascontiguousarray(AT[:, ts]),
            "atok": np.ascontiguousarray(A[ts]),
            "btok": np.ascontiguousarray(Bx[ts]),
            "cda": cda,
            "cs": cs,
            "tri": tri,
            "ids1": ids1,
            "iota": iota,
        }
        if has_b:
            bc = (b * np.float32(2.0**WS)).astype(np.float16)
            bd = ((b - bc.astype(np.float32) * np.float32(2.0**-WS)) * np.float32(2.0**WS)).astype(np.float16)
            z = np.zeros(E, np.float16)
            m["bcd"] = np.concatenate([bc, z, bd, z]).reshape(1, 4 * E)
        in_maps.append(m)

    nc = _build(has_b)
    res = run_bass_kernel_spmd(
        nc, in_maps, core_ids=list(range(NUM_CORES)), trace=TRACE
    )
    LAST_RESULTS = res

    wts = np.concatenate([r["wts"] for r in res.results], axis=1)
    gated = np.concatenate([r["gated"] for r in res.results], axis=1)
    # apply the exact repairs (pure indexing); slot value = token_id+1, 0=empty
    for c, r in enumerate(res.results):
        ids = np.rint(r["fix_ids"].reshape(-1)).astype(np.int64) - 1
        fix = r["fix_out"]
        sel = ids >= 0
        if sel.any():
            tids = ids[sel]
            assert tids.max() < TOK_PER_CORE
            gated[:, c * TOK_PER_CORE + tids] = fix[sel].T
    return (
        gated.reshape(E, Bb, S).astype(np.float32),
        wts.reshape(E, Bb, S).astype(np.float32),
    )


# revision 38
# speedup vs baseline: 1.0549x; 1.0093x over previous
"""MoE gating kernel (logits -> softmax -> top-2 mask) for 8 trn2 NeuronCores.

Math: logits = x @ W.T + b  [B,S,E]; weights = softmax(logits, -1);
gated = weights masked to per-token top-2.  Returns (gated.T, weights.T),
both [E, B, S] fp32.

Strategy (v15 = v10 hi-plane only + on-device exact repair):
  - v10 streamed a full fp16 hi/lo split of x (32 MB/core) so the top-2
    selection is exact everywhere.  But only tokens whose 2nd/3rd logit
    gap is < ~1e-3 can mis-select under hi-plane-only error (sigma
    ~1.4e-4); that's ~50 tokens per core on N(0,1) data.
  - Phase 1 streams ONLY the hi plane A = fp16(x) (16 MB/core, half the
    DMA bytes and half the PE moving passes of v10):
        logits*2^8 ~= A@C.T + A@Dp.T,  C = fp16(W*2^8), Dp = fp16(W*2^8-C)
    softmax + top-2 exactly as v10.  Weights err ~1e-4 << 2e-2 tol;
    gated can be wrong only where flagged ambiguous.
  - Per group (overlapped under the stream), tokens with max8 2nd-3rd
    gap < TAU*2^8 are flagged and COMPACTED with no data-dependent
    control flow: an intra-partition Hillis-Steele scan plus a
    strict-lower-triangular-ones matmul gives each flagged token a
    per-group rank r < 32; a one-hot is_equal against an iota grid,
    multiplied by (token_id+1) and reduced, drops id+1 into compaction
    slot [g, r] of cid_acc[128, 8, 32] (at most one nonzero per slot).
  - Phase 2 (tail): two fp32 ones-matmuls collapse cid_acc across
    partitions into a 256-slot id list (0 = empty); (id+1)-1 feeds
    per-partition-offset gpsimd indirect row gathers of the flagged
    tokens' token-major A and B = 2^11(x-A) rows (out-of-bounds skipped);
    PE transposes restore d-major and v10's exact 3-term matmul + softmax
    recomputes those <=256 tokens; fix_out/fix_ids go to DRAM and the
    host overwrites gated[:, ids] (pure indexing).
  - atok/btok (token-major planes) are uploaded but only the ~50
    gathered rows are ever read, so they cost upload time, not HW time.
"""

import functools

import numpy as np

NUM_CORES = 8
TOK_PER_CORE = 8192
GROUPS = 8
GTOK = 1024
TILES = 8
CHUNKS = 8
D = 1024
E = 16
RCAP = 32  # repair slots per group (mean ~6 flagged; P(>32) ~ 1e-12)
NSLOT = GROUPS * RCAP  # 256

XS = 11  # x = A + 2^-XS * B
WS = 8  # accumulating logits * 2^WS
TAU = 2.5e-3  # ambiguity threshold on the 2nd/3rd logit gap (unscaled)

TRACE = False
LAST_RESULTS = None


@functools.lru_cache(maxsize=2)
def _build(has_b: bool):
    from concourse import bacc, mybir
    import concourse.bass as bass
    import concourse.tile as tile
    from concourse.masks import make_identity

    f16 = mybir.dt.float16
    f32 = mybir.dt.float32
    u32 = mybir.dt.uint32
    Exp = mybir.ActivationFunctionType.Exp
    Op = mybir.AluOpType
    X = mybir.AxisListType.X

    nc = bacc.Bacc(
        "TRN2", target_bir_lowering=False, debug=False, num_devices=NUM_CORES
    )

    # A.T shard: [1024 d, 8192 t] fp16, d-major (streamed)
    at_dram = nc.dram_tensor("a_t", [D, TOK_PER_CORE], f16, kind="ExternalInput").ap()
    # token-major planes for the repair gather (only ~50 rows ever read)
    atok_dram = nc.dram_tensor("atok", [TOK_PER_CORE, D], f16, kind="ExternalInput").ap()
    btok_dram = nc.dram_tensor("btok", [TOK_PER_CORE, D], f16, kind="ExternalInput").ap()
    cda_dram = nc.dram_tensor("cda", [128, CHUNKS, 4 * E], f16, kind="ExternalInput").ap()
    cs_dram = nc.dram_tensor("cs", [128, CHUNKS, E], f16, kind="ExternalInput").ap()
    tri_dram = nc.dram_tensor("tri", [128, 128], f16, kind="ExternalInput").ap()
    ids1_dram = nc.dram_tensor("ids1", [128, GROUPS * TILES], f32, kind="ExternalInput").ap()
    iota_dram = nc.dram_tensor("iota", [128, RCAP], f32, kind="ExternalInput").ap()
    if has_b:
        bcd_dram = nc.dram_tensor("bcd", [1, 4 * E], f16, kind="ExternalInput").ap()
    wts_dram = nc.dram_tensor("wts", [E, TOK_PER_CORE], f32, kind="ExternalOutput")
    gated_dram = nc.dram_tensor("gated", [E, TOK_PER_CORE], f32, kind="ExternalOutput")
    fixids_dram = nc.dram_tensor("fix_ids", [1, NSLOT], f32, kind="ExternalOutput")
    fixout_dram = nc.dram_tensor("fix_out", [NSLOT, E], f32, kind="ExternalOutput")

    def bcast_inner(ap, n):
        return bass.AP(tensor=ap.tensor, offset=ap.offset, ap=[*ap.ap, [0, n]])

    with tile.TileContext(nc) as tc:
        with (
            tc.tile_pool(name="consts", bufs=1) as consts,
            tc.tile_pool(name="xt", bufs=3) as xt_pool,
            tc.tile_pool(name="lg", bufs=2) as lg_pool,
            tc.tile_pool(name="sm", bufs=2) as sm_pool,
            tc.tile_pool(name="oacc", bufs=1) as oacc_pool,
            tc.tile_pool(name="rep", bufs=1) as rep_pool,
            tc.tile_pool(name="pss", bufs=6, space="PSUM") as pss_pool,
            tc.tile_pool(name="pslgt", bufs=2, space="PSUM") as pslgt_pool,
        ):
            cda_sb = consts.tile([128, CHUNKS, 4 * E], f16)
            cs_sb = consts.tile([128, CHUNKS, E], f16)
            tri_sb = consts.tile([128, 128], f16)
            ids1_sb = consts.tile([128, GROUPS * TILES], f32)
            iota_sb = consts.tile([128, RCAP], f32)
            # consts go via gpsimd SWDGE so the sync HWDGE queue opens with
            # the first x-chunk instead of five table loads (ramp -3-4us)
            nc.gpsimd.dma_start(out=cda_sb, in_=cda_dram)
            nc.gpsimd.dma_start(out=cs_sb, in_=cs_dram)
            nc.gpsimd.dma_start(out=tri_sb, in_=tri_dram)
            nc.gpsimd.dma_start(out=ids1_sb, in_=ids1_dram)
            nc.gpsimd.dma_start(out=iota_sb, in_=iota_dram)
            ident32 = consts.tile([128, 128], f32)
            make_identity(nc, ident32)
            ident16 = consts.tile([128, 128], f16)
            make_identity(nc, ident16)
            ones32 = consts.tile([128, 1], f32)
            nc.vector.memset(ones32, 1.0)
            if has_b:
                bcd_sb = consts.tile([1, 4 * E], f16)
                nc.sync.dma_start(out=bcd_sb, in_=bcd_dram)
                ones_sb = consts.tile([1, 512], f16)
                nc.vector.memset(ones_sb, 1.0)

            w_acc = oacc_pool.tile([128, GROUPS, 128], f32)
            g_acc = oacc_pool.tile([128, GROUPS, 128], f32)
            flg_acc = oacc_pool.tile([128, GROUPS, TILES], f32)

            def mm_phase(g):
                xt_a = xt_pool.tile([128, CHUNKS, GTOK], f16, tag="xta")
                gs = slice(g * GTOK, (g + 1) * GTOK)
                # split loads per 2-chunk piece so matmul k can start as
                # soon as its chunks land (fine completion granularity)
                for k0 in (0, 2, 4, 6):
                    ksl = slice(k0 * 128, (k0 + 2) * 128)
                    nc.sync.dma_start(
                        out=xt_a[:, k0 : k0 + 2, :],
                        in_=at_dram[ksl, gs].rearrange("(k p) t -> p k t", p=128),
                    )

                s_h = [
                    pss_pool.tile([128, 512], f32, tag="s", name=f"s_g{g}h{h}")
                    for h in range(2)
                ]
                for k in range(CHUNKS):
                    last = k == CHUNKS - 1
                    for h in range(2):
                        ra = xt_a[:, k, 512 * h : 512 * (h + 1)]
                        nc.tensor.matmul(
                            s_h[h][0:64, :], lhsT=cda_sb[:, k, :], rhs=ra,
                            start=(k == 0), stop=(last and not has_b),
                            tile_position=(0, 0),
                        )
                if has_b:
                    for h in range(2):
                        nc.tensor.matmul(
                            s_h[h][0:64, :], lhsT=bcd_sb, rhs=ones_sb,
                            start=False, stop=True, tile_position=(0, 0),
                        )
                return s_h

            def tail_phase(g, s_h):
                # logits*2^8 = strip0 + strip32 (one PSUM input/op)
                lgS = lg_pool.tile([E, GTOK], f32, tag="lgS", name=f"lgS{g}")
                for h in range(2):
                    cmb = sm_pool.tile([E, 512], f32, tag="cmb")
                    nc.scalar.copy(cmb, s_h[h][0:16, :])
                    nc.vector.tensor_add(
                        lgS[:, 512 * h : 512 * (h + 1)], cmb, s_h[h][32:48, :]
                    )

                lgt_ps = pslgt_pool.tile([128, TILES, E], f32, tag="lgt_ps")
                for i in range(TILES):
                    nc.tensor.transpose(
                        lgt_ps[:, i, :],
                        lgS[:, 128 * i : 128 * (i + 1)],
                        ident32[:E, :E],
                    )
                lgt = sm_pool.tile([128, TILES, E], f32, tag="lgt")
                nc.vector.tensor_copy(lgt, lgt_ps)

                m8 = sm_pool.tile([128, TILES, 8], f32, tag="m8")
                for i in range(TILES):
                    nc.vector.max(m8[:, i, :], lgt[:, i, :])

                # ---- ambiguity flag (compaction itself is batched later) ----
                d23 = sm_pool.tile([128, TILES], f32, tag="d23")
                nc.vector.tensor_tensor(
                    out=d23, in0=m8[:, :, 1], in1=m8[:, :, 2], op=Op.subtract
                )
                nc.vector.tensor_scalar(
                    out=flg_acc[:, g, :], in0=d23,
                    scalar1=float(TAU * 2.0**WS), scalar2=None, op0=Op.is_lt,
                )

                # ---- softmax / top-2 / outputs (v10) ----
                ex = sm_pool.tile([128, TILES, E], f32, tag="ex")
                nc.scalar.activation(ex, lgt, func=Exp, scale=float(2.0**-WS))
                ssum = sm_pool.tile([128, TILES], f32, tag="ssum")
                nc.vector.tensor_reduce(ssum, ex, axis=X, op=Op.add)
                rec = sm_pool.tile([128, TILES], f32, tag="rec")
                nc.vector.reciprocal(rec, ssum)
                w_grp = sm_pool.tile([128, TILES, E], f32, tag="wg")
                nc.vector.tensor_tensor(
                    out=w_grp, in0=ex, in1=bcast_inner(rec[:, :], E), op=Op.mult
                )
                msk = sm_pool.tile([128, TILES, E], f32, tag="msk")
                nc.vector.tensor_tensor(
                    out=msk, in0=lgt, in1=bcast_inner(m8[:, :, 1], E), op=Op.is_ge
                )
                g_grp = sm_pool.tile([128, TILES, E], f32, tag="gg")
                nc.vector.tensor_tensor(out=g_grp, in0=msk, in1=w_grp, op=Op.mult)

                ps_o = pslgt_pool.tile([128, 256], f32, tag="lgt_ps",
                                       name=f"ps_o{g}")
                nc.tensor.transpose(ps_o[:, 0:128], w_grp, ident32)
                nc.tensor.transpose(ps_o[:, 128:256], g_grp, ident32)
                nc.scalar.copy(w_acc[:, g, :], ps_o[:, 0:128])
                nc.vector.tensor_copy(g_acc[:, g, :], ps_o[:, 128:256])

            # software pipeline: group g's matmuls, then group g-1's tail
            prev = None
            for g in range(GROUPS):
                s_h = mm_phase(g)
                if prev is not None:
                    tail_phase(prev[0], prev[1])
                prev = (g, s_h)
            tail_phase(prev[0], prev[1])

            # writeback: partition p=(tile,e); addr = e*8192 + g*1024 + tile*128 + t
            out_ap = [[128, TILES], [TOK_PER_CORE, E], [GTOK, GROUPS], [1, 128]]
            nc.sync.dma_start(
                out=bass.AP(tensor=wts_dram, offset=0, ap=list(out_ap)), in_=w_acc
            )
            nc.sync.dma_start(
                out=bass.AP(tensor=gated_dram, offset=0, ap=list(out_ap)), in_=g_acc
            )

            # ---- phase 2: batched compaction over all groups ----
            # segmented (per-group) inclusive scan along the 8 tile columns
            cur = flg_acc
            for k in (1, 2, 4):
                nxt = rep_pool.tile([128, GROUPS, TILES], f32, tag=f"bscan{k}")
                nc.vector.tensor_copy(nxt[:, :, 0:k], cur[:, :, 0:k])
                nc.vector.tensor_add(
                    nxt[:, :, k:TILES], cur[:, :, k:TILES], cur[:, :, 0 : TILES - k]
                )
                cur = nxt
            excl = rep_pool.tile([128, GROUPS, TILES], f32, tag="excl")
            nc.vector.tensor_tensor(out=excl, in0=cur, in1=flg_acc, op=Op.subtract)
            # per-group cross-partition exclusive prefix (one F=8 matmul)
            rsum16 = rep_pool.tile([128, GROUPS], f16, tag="rsum16")
            nc.vector.tensor_copy(rsum16, cur[:, :, TILES - 1])
            crps = pslgt_pool.tile([128, TILES, E], f32, tag="lgt_ps", name="crps")
            nc.tensor.matmul(crps[:, 0, 0:GROUPS], lhsT=tri_sb, rhs=rsum16,
                             start=True, stop=True)
            cross = rep_pool.tile([128, GROUPS], f32, tag="cross")
            nc.scalar.copy(cross, crps[:, 0, 0:GROUPS])
            # rank (flagged: < RCAP) or big (unflagged/overflow: no match)
            rank = rep_pool.tile([128, GROUPS, TILES], f32, tag="rank")
            nc.vector.tensor_tensor(
                out=rank, in0=excl, in1=bcast_inner(cross[:, :], TILES), op=Op.add
            )
            big = rep_pool.tile([128, GROUPS, TILES], f32, tag="big")
            nc.vector.tensor_scalar(
                out=big, in0=flg_acc, scalar1=-4096.0, scalar2=4096.0,
                op0=Op.mult, op1=Op.add,
            )
            nc.vector.tensor_add(rank, rank, big)
            # one-hot over r, times (token_id+1), reduced over tile columns
            oh = rep_pool.tile([128, GROUPS, RCAP, TILES], f32, tag="oh")
            nc.vector.tensor_tensor(
                out=oh,
                in0=bass.AP(tensor=rank.tensor, offset=rank.offset,
                            ap=[rank.ap[0], [TILES, GROUPS], [0, RCAP], [1, TILES]]),
                in1=bass.AP(tensor=iota_sb.tensor, offset=iota_sb.offset,
                            ap=[iota_sb.ap[0], [0, GROUPS], [1, RCAP], [0, TILES]]),
                op=Op.is_equal,
            )
            cidm = rep_pool.tile([128, GROUPS, RCAP, TILES], f32, tag="cidm")
            nc.vector.tensor_tensor(
                out=cidm, in0=oh,
                in1=bass.AP(tensor=ids1_sb.tensor, offset=ids1_sb.offset,
                            ap=[ids1_sb.ap[0], [TILES, GROUPS], [0, RCAP], [1, TILES]]),
                op=Op.mult,
            )
            cid_acc = rep_pool.tile([128, GROUPS, RCAP], f32, tag="cid_acc")
            nc.vector.tensor_reduce(cid_acc, cidm, axis=X, op=Op.add)

            # ---- collapse slots, gather, exact recompute ----
            cidflat = cid_acc.rearrange("p g r -> p (g r)")
            pscl = pss_pool.tile([128, 512], f32, tag="s", name="pscl")
            nc.tensor.matmul(pscl[:, 0:1], lhsT=cidflat[:, 0:128], rhs=ones32,
                             start=True, stop=True)
            nc.tensor.matmul(pscl[:, 1:2], lhsT=cidflat[:, 128:256], rhs=ones32,
                             start=True, stop=True)
            cidL = rep_pool.tile([128, 2], f32, tag="cidL")
            nc.scalar.copy(cidL, pscl[:, 0:2])
            nc.sync.dma_start(
                out=bass.AP(tensor=fixids_dram, offset=0, ap=[[1, 128], [128, 2]]),
                in_=cidL,
            )
            idm1 = rep_pool.tile([128, 2], f32, tag="idm1")
            nc.vector.tensor_scalar(
                out=idm1, in0=cidL, scalar1=-1.0, scalar2=None, op0=Op.add
            )
            idu = rep_pool.tile([128, 2], u32, tag="idu")
            nc.vector.tensor_copy(idu, idm1)

            # per-partition-offset row gathers; empty slots (-1 -> huge) skipped
            xra = rep_pool.tile([128, 2, D], f16, tag="xra")
            xrb = rep_pool.tile([128, 2, D], f16, tag="xrb")
            for c in range(2):
                nc.gpsimd.indirect_dma_start(
                    out=xra[:, c, :], out_offset=None, in_=atok_dram,
                    in_offset=bass.IndirectOffsetOnAxis(ap=idu[:, c : c + 1], axis=0),
                    bounds_check=TOK_PER_CORE - 1, oob_is_err=False,
                )
                nc.gpsimd.indirect_dma_start(
                    out=xrb[:, c, :], out_offset=None, in_=btok_dram,
                    in_offset=bass.IndirectOffsetOnAxis(ap=idu[:, c : c + 1], axis=0),
                    bounds_check=TOK_PER_CORE - 1, oob_is_err=False,
                )

            # token-major -> d-major via f16 PE transposes (no f32 roundtrip)
            xta = rep_pool.tile([128, CHUNKS, NSLOT], f16, tag="xta2")
            xtb = rep_pool.tile([128, CHUNKS, NSLOT], f16, tag="xtb2")
            for srcg, dst in ((xra, xta), (xrb, xtb)):
                for c in range(2):
                    for k in range(CHUNKS):
                        pst = pss_pool.tile([128, 512], f16, tag="s",
                                            name=f"pst{c}_{k}")
                        nc.tensor.transpose(
                            pst[:, 0:128],
                            srcg[:, c, 128 * k : 128 * (k + 1)], ident16,
                        )
                        nc.vector.tensor_copy(
                            dst[:, k, 128 * c : 128 * (c + 1)], pst[:, 0:128]
                        )

            # exact 3-term matmul for the repaired tokens (v10 math)
            s2 = pss_pool.tile([128, 512], f32, tag="s", name="s2rep")
            for k in range(CHUNKS):
                last = k == CHUNKS - 1
                nc.tensor.matmul(
                    s2[0:64, 0:NSLOT], lhsT=cda_sb[:, k, :], rhs=xta[:, k, :],
                    start=(k == 0), stop=(last and not has_b), tile_position=(0, 0),
                )
                nc.tensor.matmul(
                    s2[64:80, 0:NSLOT], lhsT=cs_sb[:, k, :], rhs=xtb[:, k, :],
                    start=(k == 0), stop=(last and not has_b), tile_position=(0, 64),
                )
            if has_b:
                nc.tensor.matmul(
                    s2[0:64, 0:NSLOT], lhsT=bcd_sb, rhs=ones_sb[:, 0:NSLOT],
                    start=False, stop=True, tile_position=(0, 0),
                )
                nc.tensor.matmul(
                    s2[64:80, 0:NSLOT], lhsT=cs_sb[0:1, 0, :], rhs=ones_sb[:, 0:NSLOT],
                    start=False, stop=True, tile_position=(0, 64),
                    skip_group_check=True,
                )

            c2 = rep_pool.tile([E, NSLOT], f32, tag="c2")
            nc.scalar.copy(c2, s2[0:16, 0:NSLOT])
            l2 = rep_pool.tile([E, NSLOT], f32, tag="l2")
            nc.vector.tensor_add(l2, c2, s2[32:48, 0:NSLOT])
            lr2 = rep_pool.tile([E, NSLOT], f32, tag="lr2")
            nc.vector.tensor_add(lr2, l2, s2[64:80, 0:NSLOT])

            psl = pslgt_pool.tile([128, TILES, E], f32, tag="lgt_ps", name="pslrep")
            for c in range(2):
                nc.tensor.transpose(
                    psl[:, c, :], lr2[:, 128 * c : 128 * (c + 1)], ident32[:E, :E]
                )
            lgr = rep_pool.tile([128, 2, E], f32, tag="lgr")
            nc.vector.tensor_copy(lgr, psl[:, 0:2, :])

            m8r = rep_pool.tile([128, 2, 8], f32, tag="m8r")
            for c in range(2):
                nc.vector.max(m8r[:, c, :], lgr[:, c, :])
            exr = rep_pool.tile([128, 2, E], f32, tag="exr")
            nc.scalar.activation(exr, lgr, func=Exp, scale=float(2.0**-WS))
            ssr = rep_pool.tile([128, 2], f32, tag="ssr")
            nc.vector.tensor_reduce(ssr, exr, axis=X, op=Op.add)
            rcr = rep_pool.tile([128, 2], f32, tag="rcr")
            nc.vector.reciprocal(rcr, ssr)
            wr = rep_pool.tile([128, 2, E], f32, tag="wr")
            nc.vector.tensor_tensor(
                out=wr, in0=exr, in1=bcast_inner(rcr[:, :], E), op=Op.mult
            )
            mkr = rep_pool.tile([128, 2, E], f32, tag="mkr")
            nc.vector.tensor_tensor(
                out=mkr, in0=lgr, in1=bcast_inner(m8r[:, :, 1], E), op=Op.is_ge
            )
            gr = rep_pool.tile([128, 2, E], f32, tag="gr")
            nc.vector.tensor_tensor(out=gr, in0=mkr, in1=wr, op=Op.mult)
            nc.sync.dma_start(
                out=bass.AP(tensor=fixout_dram, offset=0,
                            ap=[[E, 128], [128 * E, 2], [1, E]]),
                in_=gr,
            )

    nc.compile()
    return nc


def _w_consts(W):
    C = (W * np.float32(2.0**WS)).astype(np.float16)
    Dp = ((W - C.astype(np.float32) * np.float32(2.0**-WS)) * np.float32(2.0**WS)).astype(np.float16)
    Cs = (C.astype(np.float32) * np.float32(2.0**-XS)).astype(np.float16)

    def lay(M):  # [16, 1024] -> [128 d_lo, chunks, E]
        return np.ascontiguousarray(M.T.reshape(CHUNKS, 128, E).transpose(1, 0, 2))

    cda = np.zeros((128, CHUNKS, 4 * E), np.float16)
    cda[:, :, 0:E] = lay(C)
    cda[:, :, 2 * E : 3 * E] = lay(Dp)
    return cda, lay(Cs)


def kernel(x, W, b):
    global LAST_RESULTS
    from concourse.bass_utils import run_bass_kernel_spmd

    x = np.ascontiguousarray(np.asarray(x, dtype=np.float32))
    W = np.ascontiguousarray(np.asarray(W, dtype=np.float32))
    b = np.ascontiguousarray(np.asarray(b, dtype=np.float32))
    Bb, S, Dd = x.shape
    ntok = Bb * S
    assert (ntok, Dd) == (NUM_CORES * TOK_PER_CORE, D) and W.shape == (E, D)

    xf = x.reshape(ntok, D)
    A = xf.astype(np.float16)
    Bx = ((xf - A.astype(np.float32)) * np.float32(2.0**XS)).astype(np.float16)
    AT = np.ascontiguousarray(A.T)  # [1024, 65536]

    cda, cs = _w_consts(W)
    tri = np.fromfunction(lambda a, c: a < c, (128, 128)).astype(np.float16)
    j = np.arange(GROUPS * TILES)
    ids1 = ((j // TILES) * GTOK + (j % TILES) * 128)[None, :] + np.arange(128)[:, None]
    ids1 = (ids1 + 1).astype(np.float32)  # 0 stays "empty slot"
    iota = np.tile(np.arange(RCAP, dtype=np.float32), (128, 1))

    has_b = bool(np.any(b))
    in_maps = []
    for c in range(NUM_CORES):
        ts = slice(c * TOK_PER_CORE, (c + 1) * TOK_PER_CORE)
        m = {
            "a_t": np.ascontiguousarray(AT[:, ts]),
            "atok": np.ascontiguousarray(A[ts]),
            "btok": np.ascontiguousarray(Bx[ts]),
            "cda": cda,
            "cs": cs,
            "tri": tri,
            "ids1": ids1,
            "iota": iota,
        }
        if has_b:
            bc = (b * np.float32(2.0**WS)).astype(np.float16)
            bd = ((b - bc.astype(np.float32) * np.float32(2.0**-WS)) * np.float32(2.0**WS)).astype(np.float16)
            z = np.zeros(E, np.float16)
            m["bcd"] = np.concatenate([bc, z, bd, z]).reshape(1, 4 * E)
        in_maps.append(m)

    nc = _build(has_b)
    res = run_bass_kernel_spmd(
        nc, in_maps, core_ids=list(range(NUM_CORES)), trace=TRACE
    )
    LAST_RESULTS = res

    wts = np.concatenate([r["wts"] for r in res.results], axis=1)
    gated = np.concatenate([r["gated"] for r in res.results], axis=1)
    # apply the exact repairs (pure indexing); slot value = token_id+1, 0=empty
    for c, r in enumerate(res.results):
        ids = np.rint(r["fix_ids"].reshape(-1)).astype(np.int64) - 1
        fix = r["fix_out"]
        sel = ids >= 0
        if sel.any():
            tids = ids[sel]
            assert tids.max() < TOK_PER_CORE
            gated[:, c * TOK_PER_CORE + tids] = fix[sel].T
    return (
        gated.reshape(E, Bb, S).astype(np.float32),
        wts.reshape(E, Bb, S).astype(np.float32),
    )
